# revision 33
# baseline (speedup 1.0000x reference)
"""Trainium2 Bass kernel for nn_EqualtimeLayer (spiking-neuron time-to-first-spike).

Math: for each (batch b, postsyn j) the output is the earliest T where
    f(T) = sum_i w[i,j] * relu(T - t[i,j]) >= theta_j,   t[i,j] = s[b,i] + d[i,j]
(first upward threshold crossing of the linear-PSP membrane potential; equivalent
to the reference's sort+cumsum+first-valid-window computation).

Device algorithm (no sort needed):
    S(tau) = sum_i w*max(t,tau)
    f(tau) >= theta  <=>  S(tau) >= thW := theta + sum_i w*t
    -> 4 rounds of bisection on [0,2], then 2 false-position probes, then a
       final computed false-position candidate. The bracket endpoints' S values
       start analytically known: S(0) = sum w*t = thW - theta, S(2) = 2*sum w
       (all t < 2), so false position needs no slope probes.

Each probe is evaluated with a two-lane engine split (the DVE fused
scalar_tensor_tensor runs at 1x only; tensor_scalar hits the 4x perf mode):
  - ACT-lane columns (24 = batches 0-2): DVE tensor_scalar(max) [4x] into the
    two halves of a [128, 2048] pair tile, one tensor_tensor(mult) [2x] over
    the pair (amortizes the per-op init), and the otherwise-idle Activation
    engine accumulates each half (activation Copy with accum_out).
  - DVE-lane columns (8 = batch 3): single fused scalar_tensor_tensor (1x).
Data is fp16 (DVE 2x/4x modes need 2-byte dtypes); state/accums are fp32. thW
is computed on the host from the same fp16-rounded inputs for consistency.

Sharding: data-parallel over batch, 4 batches per core on 8 cores. Weights and
delays are transposed once on the host (j-major layout) so probes are
per-partition-scalar ops with j on partitions and i on the free axis.
"""

import numpy as np

import concourse.bacc as bacc
import concourse.mybir as mybir
import concourse.tile as tile
from concourse.bass_utils import run_bass_kernel_spmd

F32 = mybir.dt.float32
F16 = mybir.dt.float16
U8 = mybir.dt.uint8
ALU = mybir.AluOpType
ACTF = mybir.ActivationFunctionType

B, PRE, POST = 32, 1024, 1024
N_CORES = 8
B_LOC = B // N_CORES          # 4 batches per core
JB = POST // 128              # 8 j-blocks of 128 partitions
NCOL = B_LOC * JB             # 32 state columns, col = b*JB + jb
R_BISECT = 4                  # bisection rounds
R_FALSEPOS = 2                # false-position probe rounds
N_ACT = 22                    # columns on the ACT lane (pairs; rest DVE-fused)


def _build(n_bisect=R_BISECT, n_fp=R_FALSEPOS, infguard=True):
    nc = bacc.Bacc("TRN2", target_bir_lowering=False, debug=False)

    dT = nc.dram_tensor("dT", [POST, PRE], F16, kind="ExternalInput")      # d transposed [j, i]
    wT = nc.dram_tensor("wT", [POST, PRE], F16, kind="ExternalInput")      # w transposed [j, i]
    s_loc = nc.dram_tensor("s_loc", [B_LOC, 2 * PRE], F16, kind="ExternalInput")   # s duplicated 2x
    # state-layout inputs/outputs: [128, NCOL] with col = b*JB + jb, j = jb*128+p
    # (host pre-/post-permutes; keeps every DMA a clean contiguous 2D copy)
    thw_in = nc.dram_tensor("thw_in", [128, NCOL], F32, kind="ExternalInput")
    slo_in = nc.dram_tensor("slo_in", [128, NCOL], F32, kind="ExternalInput")   # S(0) = thW - theta
    shi_in = nc.dram_tensor("shi_in", [128, NCOL], F32, kind="ExternalInput")   # S(2) = 2*sum w
    out_loc = nc.dram_tensor("out_loc", [128, NCOL], F32, kind="ExternalOutput")

    n_act = N_ACT

    with tile.TileContext(nc) as tc:
        with (
            tc.tile_pool(name="big", bufs=1) as big,
            tc.tile_pool(name="mat", bufs=1) as mat,
            tc.tile_pool(name="midp", bufs=2) as midp,
            tc.tile_pool(name="prodp", bufs=6) as prodp,
            tc.tile_pool(name="small", bufs=1) as small,
        ):
            # ---- load d^T and w^T, build t^T[b,jb] = d^T + s[b] ----
            # sync queue: dT0, sreps, dT1..7 (build-order critical path);
            # scalar queue: all wT + state tiles in parallel.
            NQ = JB // 2
            dw = [mat.tile([128, 2 * PRE], F16, tag=f"dw{q}", name=f"dT{q}") for q in range(NQ)]
            sreps = []
            for b in range(B_LOC):
                srep = midp.tile([128, 2 * PRE], F16, tag=f"srep{b % 2}", name=f"srep{b}")
                nc.scalar.dma_start(out=srep[:], in_=s_loc[b:b + 1, :].partition_broadcast(128))
                sreps.append(srep)
            for q in range(NQ):
                for h in range(2):
                    jb = 2 * q + h
                    nc.sync.dma_start(out=dw[q][:, h * PRE:(h + 1) * PRE],
                                      in_=dT[jb * 128:(jb + 1) * 128, :])

            # all w slices in ONE tile so a product op can span adjacent jb's
            w_all = mat.tile([128, JB * PRE], F16, tag="w_all", name="w_all")
            for jb in range(JB):
                nc.scalar.dma_start(out=w_all[:, jb * PRE:(jb + 1) * PRE],
                                    in_=wT[jb * 128:(jb + 1) * 128, :])

            # paired t tiles: one [128, 2*PRE] tile per (b, jb-pair)
            tT2 = {}
            for b in range(B_LOC):
                for q in range(NQ):
                    tT2[(b, q)] = big.tile([128, 2 * PRE], F16, tag=f"tT{b}_{q}",
                                           name=f"tT{b}_{q}")

            def t_slice(b, jb):
                return tT2[(b, jb // 2)][:, (jb % 2) * PRE:(jb % 2 + 1) * PRE]

            # ---- per-column state, col = b*JB + jb ----
            def st(tag, dt=F32, ncol=NCOL):
                return small.tile([128, ncol], dt, tag=tag, name=tag)

            lo, hi, tau, thW = st("lo"), st("hi"), st("tau"), st("thW")
            S_lo, S_hi = st("S_lo"), st("S_hi")
            # S is split by accumulating engine to keep single-writer tiles
            S_act = st("S_act", ncol=n_act)
            S_dve = st("S_dve", ncol=NCOL - n_act)
            pred_ge, pred_lt = st("pge", U8), st("plt", U8)
            scr0, scr1, scr2 = st("scr0"), st("scr1"), st("scr2")
            dump = midp.tile([128, PRE], F16, tag="dump", name="dump")

            QW = 2  # ACT-lane columns grouped per product op

            def probe_group(b, jbq, scalar_tile):
                """ACT-lane probe of cols (b, QW*jbq .. QW*jbq+QW-1): QW 4x
                tensor_scalar maxes into a [128, QW*PRE] group, one 2x
                tensor_tensor mult over the group, QW ACT accumulations."""
                jbs = [QW * jbq + i for i in range(QW)]
                cols = [b * JB + jb for jb in jbs]
                mp = midp.tile([128, QW * PRE], F16, tag=f"mp{(b + jbq) % 2}", name="mp")
                for i, (jb, c) in enumerate(zip(jbs, cols)):
                    nc.vector.tensor_scalar(
                        out=mp[:, i * PRE:(i + 1) * PRE], in0=t_slice(b, jb),
                        scalar1=scalar_tile[:, c:c + 1], scalar2=None, op0=ALU.max)
                prod = prodp.tile([128, QW * PRE], F16, tag="prod", name="prod")
                nc.vector.tensor_tensor(
                    out=prod[:], in0=mp[:],
                    in1=w_all[:, jbs[0] * PRE:(jbs[-1] + 1) * PRE], op=ALU.mult)
                for i, c in enumerate(cols):
                    nc.scalar.activation(
                        out=dump[:], in_=prod[:, i * PRE:(i + 1) * PRE],
                        func=ACTF.Copy, scale=1.0,
                        accum_out=S_act[:, c:c + 1])

            def probe_stt(b, jb, scalar_tile):
                col = b * JB + jb
                scratch = prodp.tile([128, PRE], F16, tag="sc16", name="scratch")
                nc.vector.scalar_tensor_tensor(
                    out=scratch[:],
                    in0=t_slice(b, jb),
                    scalar=scalar_tile[:, col:col + 1],
                    in1=w_all[:, jb * PRE:(jb + 1) * PRE],
                    op0=ALU.max, op1=ALU.mult,
                    accum_out=S_dve[:, col - n_act:col - n_act + 1])

            def probe(scalar_tile, with_build=False):
                for jbq in range(JB // QW):
                    for b in range(B_LOC):
                        if with_build:
                            nc.vector.tensor_tensor(
                                out=tT2[(b, jbq)][:], in0=dw[jbq][:],
                                in1=sreps[b][:], op=ALU.add)
                        if b * JB + QW * jbq + QW <= n_act:
                            probe_group(b, jbq, scalar_tile)
                        else:
                            for jb in range(QW * jbq, QW * jbq + QW):
                                probe_stt(b, jb, scalar_tile)

            # state-layout loads (already permuted on the host)
            for (dram, sb_tile) in ((thw_in, thW), (slo_in, S_lo), (shi_in, S_hi)):
                nc.scalar.dma_start(out=sb_tile[:], in_=dram[:, :])

            nc.vector.memset(lo[:], 0.0)
            nc.vector.memset(hi[:], 2.0)

            def falsepos_tau(out_tile, clip_interior):
                """out = lo + (thW - S_lo)*(hi - lo)/(S_hi - S_lo), safeguarded."""
                nc.vector.tensor_tensor(out=scr0[:], in0=S_hi[:], in1=S_lo[:], op=ALU.subtract)
                nc.vector.reciprocal(out=scr1[:], in_=scr0[:])
                nc.vector.tensor_tensor(out=scr2[:], in0=thW[:], in1=S_lo[:], op=ALU.subtract)
                nc.vector.tensor_tensor(out=scr1[:], in0=scr2[:], in1=scr1[:], op=ALU.mult)
                nc.vector.tensor_tensor(out=scr2[:], in0=hi[:], in1=lo[:], op=ALU.subtract)
                nc.vector.tensor_tensor(out=scr1[:], in0=scr1[:], in1=scr2[:], op=ALU.mult)
                nc.vector.tensor_tensor(out=out_tile[:], in0=scr1[:], in1=lo[:], op=ALU.add)
                if clip_interior:
                    nc.vector.tensor_scalar_mul(scr1[:], scr2[:], 0.02)
                    nc.vector.tensor_tensor(out=scr2[:], in0=lo[:], in1=scr1[:], op=ALU.add)
                    nc.vector.tensor_tensor(out=out_tile[:], in0=out_tile[:], in1=scr2[:], op=ALU.max)
                    nc.vector.tensor_tensor(out=scr2[:], in0=hi[:], in1=scr1[:], op=ALU.subtract)
                    nc.vector.tensor_tensor(out=out_tile[:], in0=out_tile[:], in1=scr2[:], op=ALU.min)
                else:
                    nc.vector.tensor_tensor(out=out_tile[:], in0=out_tile[:], in1=lo[:], op=ALU.max)
                    nc.vector.tensor_tensor(out=out_tile[:], in0=out_tile[:], in1=hi[:], op=ALU.min)
                # guard: if S_hi - S_lo <= 0 fall back to midpoint
                nc.vector.tensor_scalar(out=pred_lt[:], in0=scr0[:], scalar1=0.0, scalar2=None,
                                        op0=ALU.is_le)
                nc.vector.tensor_tensor(out=scr2[:], in0=lo[:], in1=hi[:], op=ALU.add)
                nc.vector.tensor_scalar_mul(scr2[:], scr2[:], 0.5)
                nc.vector.copy_predicated(out=out_tile[:], mask=pred_lt[:], data=scr2[:])

            for k in range(n_bisect + n_fp):
                if k < n_bisect:
                    nc.vector.tensor_tensor(out=scr0[:], in0=lo[:], in1=hi[:], op=ALU.add)
                    nc.vector.tensor_scalar_mul(tau[:], scr0[:], 0.5)
                else:
                    falsepos_tau(tau, clip_interior=True)
                # round 0 fuses the t-build into the probe stream
                probe(tau, with_build=(k == 0))
                # bracket update: DVE-lane columns first (their accums land
                # before the ACT accumulator tail, so DVE does useful work
                # instead of stalling on S_act)
                nc.vector.tensor_tensor(out=pred_ge[:, n_act:], in0=S_dve[:],
                                        in1=thW[:, n_act:], op=ALU.is_ge)
                nc.vector.tensor_tensor(out=pred_lt[:, n_act:], in0=S_dve[:],
                                        in1=thW[:, n_act:], op=ALU.is_lt)
                nc.vector.copy_predicated(out=hi[:, n_act:], mask=pred_ge[:, n_act:], data=tau[:, n_act:])
                nc.vector.copy_predicated(out=lo[:, n_act:], mask=pred_lt[:, n_act:], data=tau[:, n_act:])
                nc.vector.copy_predicated(out=S_hi[:, n_act:], mask=pred_ge[:, n_act:], data=S_dve[:])
                nc.vector.copy_predicated(out=S_lo[:, n_act:], mask=pred_lt[:, n_act:], data=S_dve[:])
                nc.vector.tensor_tensor(out=pred_ge[:, :n_act], in0=S_act[:],
                                        in1=thW[:, :n_act], op=ALU.is_ge)
                nc.vector.tensor_tensor(out=pred_lt[:, :n_act], in0=S_act[:],
                                        in1=thW[:, :n_act], op=ALU.is_lt)
                nc.vector.copy_predicated(out=hi[:, :n_act], mask=pred_ge[:, :n_act], data=tau[:, :n_act])
                nc.vector.copy_predicated(out=lo[:, :n_act], mask=pred_lt[:, :n_act], data=tau[:, :n_act])
                nc.vector.copy_predicated(out=S_hi[:, :n_act], mask=pred_ge[:, :n_act], data=S_act[:])
                nc.vector.copy_predicated(out=S_lo[:, :n_act], mask=pred_lt[:, :n_act], data=S_act[:])

            # ---- final computed candidate (no probe) ----
            cand = st("cand")
            falsepos_tau(cand, clip_interior=False)
            if infguard:
                infs = st("infs")
                nc.vector.memset(infs[:], float("inf"))
                nc.vector.tensor_scalar(out=pred_ge[:], in0=hi[:], scalar1=2.0, scalar2=None,
                                        op0=ALU.is_ge)
                nc.vector.copy_predicated(out=cand[:], mask=pred_ge[:], data=infs[:])

            nc.sync.dma_start(out=out_loc[:, :], in_=cand[:])

    nc.compile()
    return nc


_NC_CACHE = None


def _host_prep(input_spikes, input_weights, input_delays, thresholds):
    s16_1 = np.asarray(input_spikes, dtype=np.float16)
    s16 = np.ascontiguousarray(np.concatenate([s16_1, s16_1], axis=1))
    _sb1 = s16_1.astype(np.float32)
    w16 = np.asarray(input_weights, dtype=np.float16)
    d16 = np.asarray(input_delays, dtype=np.float16)
    th = np.asarray(thresholds, dtype=np.float32)
    sb = _sb1
    wb = w16.astype(np.float32)
    db = d16.astype(np.float32)
    thw = (th[None, :] + (wb * db).sum(axis=0, dtype=np.float32)[None, :]
           + sb @ wb).astype(np.float32)
    slo = (thw - th[None, :]).astype(np.float32)                       # S(0)
    shi = np.broadcast_to(2.0 * wb.sum(axis=0, dtype=np.float32),      # S(2)
                          thw.shape).astype(np.float32)
    wT = np.ascontiguousarray(w16.T)
    dT = np.ascontiguousarray(d16.T)
    return s16, wT, dT, thw, slo, shi


def _to_state_layout(x_bloc_post):
    """[B_LOC, POST] -> [128, NCOL] with col = b*JB + jb, j = jb*128 + p."""
    x = np.asarray(x_bloc_post).reshape(B_LOC, JB, 128)        # [b, jb, p]
    return np.ascontiguousarray(x.transpose(2, 0, 1).reshape(128, NCOL))


def _from_state_layout(x_128_ncol):
    """[128, NCOL] -> [B_LOC, POST]."""
    x = np.asarray(x_128_ncol).reshape(128, B_LOC, JB)         # [p, b, jb]
    return np.ascontiguousarray(x.transpose(1, 2, 0).reshape(B_LOC, POST))


def kernel(input_spikes, input_weights, input_delays, thresholds):
    global _NC_CACHE
    if _NC_CACHE is None:
        _NC_CACHE = _build()
    nc = _NC_CACHE

    s16, wT, dT, thw, slo, shi = _host_prep(
        input_spikes, input_weights, input_delays, thresholds)

    in_maps = [
        dict(dT=dT, wT=wT,
             s_loc=np.ascontiguousarray(s16[k * B_LOC:(k + 1) * B_LOC]),
             thw_in=_to_state_layout(thw[k * B_LOC:(k + 1) * B_LOC]),
             slo_in=_to_state_layout(slo[k * B_LOC:(k + 1) * B_LOC]),
             shi_in=_to_state_layout(shi[k * B_LOC:(k + 1) * B_LOC]))
        for k in range(N_CORES)
    ]
    res = run_bass_kernel_spmd(nc, in_maps, core_ids=list(range(N_CORES)))
    out = np.concatenate([_from_state_layout(r["out_loc"]) for r in res.results],
                         axis=0)
    return out.astype(np.float32)


if __name__ == "__main__":
    rng = np.random.default_rng(0)
    s = rng.uniform(0, 1, (B, PRE)).astype(np.float32)
    w = (rng.normal(0, 1, (PRE, POST)) * 0.1 + 0.05).astype(np.float32)
    d = rng.uniform(0, 1, (PRE, POST)).astype(np.float32)
    th = np.ones(POST, np.float32)
    out = kernel(s, w, d, th)
    print("out", out.shape, out.dtype, np.percentile(out[np.isfinite(out)], [0, 50, 100]))


# revision 34
# speedup vs baseline: 1.1884x; 1.1884x over previous
"""Trainium2 Bass kernel for nn_EqualtimeLayer (spiking-neuron time-to-first-spike).

Math: for each (batch b, postsyn j) the output is the earliest T where
    f(T) = sum_i w[i,j] * relu(T - t[i,j]) >= theta_j,   t[i,j] = s[b,i] + d[i,j]
(first upward threshold crossing of the linear-PSP membrane potential; equivalent
to the reference's sort+cumsum+first-valid-window computation).

Device algorithm (no sort needed):
    S(tau) = sum_i w*max(t,tau)
    f(tau) >= theta  <=>  S(tau) >= thW := theta + sum_i w*t
    -> 4 rounds of bisection on [0,2], then 2 false-position probes, then a
       final computed false-position candidate. The bracket endpoints' S values
       start analytically known: S(0) = sum w*t = thW - theta, S(2) = 2*sum w
       (all t < 2), so false position needs no slope probes.

Each probe is evaluated with a two-lane engine split (the DVE fused
scalar_tensor_tensor runs at 1x only; tensor_scalar hits the 4x perf mode):
  - ACT-lane columns (24 = batches 0-2): DVE tensor_scalar(max) [4x] into the
    two halves of a [128, 2048] pair tile, one tensor_tensor(mult) [2x] over
    the pair (amortizes the per-op init), and the otherwise-idle Activation
    engine accumulates each half (activation Copy with accum_out).
  - DVE-lane columns (8 = batch 3): single fused scalar_tensor_tensor (1x).
Data is fp16 (DVE 2x/4x modes need 2-byte dtypes); state/accums are fp32. thW
is computed on the host from the same fp16-rounded inputs for consistency.

Sharding: data-parallel over batch, 4 batches per core on 8 cores. Weights and
delays are transposed once on the host (j-major layout) so probes are
per-partition-scalar ops with j on partitions and i on the free axis.
"""

import numpy as np

import concourse.bacc as bacc
import concourse.mybir as mybir
import concourse.tile as tile
from concourse.bass_utils import run_bass_kernel_spmd

F32 = mybir.dt.float32
F16 = mybir.dt.float16
U8 = mybir.dt.uint8
ALU = mybir.AluOpType
ACTF = mybir.ActivationFunctionType

B, PRE, POST = 32, 1024, 1024
N_CORES = 8
B_LOC = B // N_CORES          # 4 batches per core
JB = POST // 128              # 8 j-blocks of 128 partitions
NCOL = B_LOC * JB             # 32 state columns, col = b*JB + jb
R_BISECT = 4                  # bisection rounds
R_FALSEPOS = 2                # false-position probe rounds
N_ACT = 22                    # columns on the ACT lane (pairs; rest DVE-fused)


def _build(n_bisect=R_BISECT, n_fp=R_FALSEPOS, infguard=True):
    nc = bacc.Bacc("TRN2", target_bir_lowering=False, debug=False)

    dT = nc.dram_tensor("dT", [POST, PRE], F16, kind="ExternalInput")      # d transposed [j, i]
    wT = nc.dram_tensor("wT", [POST, PRE], F16, kind="ExternalInput")      # w transposed [j, i]
    s_loc = nc.dram_tensor("s_loc", [B_LOC, PRE], F16, kind="ExternalInput")
    # state-layout inputs/outputs: [128, NCOL] with col = b*JB + jb, j = jb*128+p
    # (host pre-/post-permutes; keeps every DMA a clean contiguous 2D copy)
    thw_in = nc.dram_tensor("thw_in", [128, NCOL], F32, kind="ExternalInput")
    slo_in = nc.dram_tensor("slo_in", [128, NCOL], F32, kind="ExternalInput")   # S(0) = thW - theta
    shi_in = nc.dram_tensor("shi_in", [128, NCOL], F32, kind="ExternalInput")   # S(2) = 2*sum w
    out_loc = nc.dram_tensor("out_loc", [128, NCOL], F32, kind="ExternalOutput")

    n_act = N_ACT

    with tile.TileContext(nc) as tc:
        with (
            tc.tile_pool(name="big", bufs=1) as big,
            tc.tile_pool(name="mat", bufs=1) as mat,
            tc.tile_pool(name="midp", bufs=2) as midp,
            tc.tile_pool(name="prodp", bufs=6) as prodp,
            tc.tile_pool(name="small", bufs=1) as small,
        ):
            # ---- load d^T and w^T, build t^T[b,jb] = d^T + s[b] ----
            # sync queue: dT0, sreps, dT1..7 (build-order critical path);
            # scalar queue: all wT + state tiles in parallel.
            dw = [mat.tile([128, PRE], F16, tag=f"dw{jb}", name=f"dT{jb}") for jb in range(JB)]
            sreps = []
            for b in range(B_LOC):
                srep = midp.tile([128, PRE], F16, tag=f"srep{b % 2}", name=f"srep{b}")
                nc.scalar.dma_start(out=srep[:], in_=s_loc[b:b + 1, :].partition_broadcast(128))
                sreps.append(srep)
            for jb in range(JB):
                nc.sync.dma_start(out=dw[jb][:], in_=dT[jb * 128:(jb + 1) * 128, :])

            # all w slices in ONE tile so a product op can span adjacent jb's
            w_all = mat.tile([128, JB * PRE], F16, tag="w_all", name="w_all")
            for jb in range(JB):
                nc.scalar.dma_start(out=w_all[:, jb * PRE:(jb + 1) * PRE],
                                    in_=wT[jb * 128:(jb + 1) * 128, :])

            # flat 2D t tiles, except ACT-lane batches keep per-b pair tiles
            tT = {}
            for b in range(B_LOC):
                for jb in range(JB):
                    tT[(b, jb)] = big.tile([128, PRE], F16, tag=f"tT{b}_{jb}",
                                           name=f"tT{b}_{jb}")

            # ---- per-column state, col = b*JB + jb ----
            def st(tag, dt=F32, ncol=NCOL):
                return small.tile([128, ncol], dt, tag=tag, name=tag)

            lo, hi, tau, thW = st("lo"), st("hi"), st("tau"), st("thW")
            S_lo, S_hi = st("S_lo"), st("S_hi")
            # S is split by accumulating engine to keep single-writer tiles
            S_act = st("S_act", ncol=n_act)
            S_dve = st("S_dve", ncol=NCOL - n_act)
            pred_ge, pred_lt = st("pge", U8), st("plt", U8)
            scr0, scr1, scr2 = st("scr0"), st("scr1"), st("scr2")
            dump = midp.tile([128, PRE], F16, tag="dump", name="dump")

            QW = 2  # ACT-lane columns grouped per product op

            def probe_group(b, jbq, scalar_tile):
                """ACT-lane probe of cols (b, QW*jbq .. QW*jbq+QW-1): QW 4x
                tensor_scalar maxes into a [128, QW*PRE] group, one 2x
                tensor_tensor mult over the group, QW ACT accumulations."""
                jbs = [QW * jbq + i for i in range(QW)]
                cols = [b * JB + jb for jb in jbs]
                mp = midp.tile([128, QW * PRE], F16, tag=f"mp{(b + jbq) % 2}", name="mp")
                for i, (jb, c) in enumerate(zip(jbs, cols)):
                    nc.vector.tensor_scalar(
                        out=mp[:, i * PRE:(i + 1) * PRE], in0=tT[(b, jb)][:],
                        scalar1=scalar_tile[:, c:c + 1], scalar2=None, op0=ALU.max)
                prod = prodp.tile([128, QW * PRE], F16, tag="prod", name="prod")
                nc.vector.tensor_tensor(
                    out=prod[:], in0=mp[:],
                    in1=w_all[:, jbs[0] * PRE:(jbs[-1] + 1) * PRE], op=ALU.mult)
                for i, c in enumerate(cols):
                    nc.scalar.activation(
                        out=dump[:], in_=prod[:, i * PRE:(i + 1) * PRE],
                        func=ACTF.Copy, scale=1.0,
                        accum_out=S_act[:, c:c + 1])

            def probe_stt(b, jb, scalar_tile):
                col = b * JB + jb
                scratch = prodp.tile([128, PRE], F16, tag="sc16", name="scratch")
                nc.vector.scalar_tensor_tensor(
                    out=scratch[:],
                    in0=tT[(b, jb)][:],
                    scalar=scalar_tile[:, col:col + 1],
                    in1=w_all[:, jb * PRE:(jb + 1) * PRE],
                    op0=ALU.max, op1=ALU.mult,
                    accum_out=S_dve[:, col - n_act:col - n_act + 1])

            def probe(scalar_tile, with_build=False):
                for jbq in range(JB // QW):
                    for b in range(B_LOC):
                        if with_build:
                            for jb in range(QW * jbq, QW * jbq + QW):
                                nc.vector.tensor_tensor(
                                    out=tT[(b, jb)][:], in0=dw[jb][:],
                                    in1=sreps[b][:], op=ALU.add)
                        if b * JB + QW * jbq + QW <= n_act:
                            probe_group(b, jbq, scalar_tile)
                        else:
                            for jb in range(QW * jbq, QW * jbq + QW):
                                probe_stt(b, jb, scalar_tile)

            # state-layout loads (already permuted on the host)
            for (dram, sb_tile) in ((thw_in, thW), (slo_in, S_lo), (shi_in, S_hi)):
                nc.scalar.dma_start(out=sb_tile[:], in_=dram[:, :])

            nc.vector.memset(lo[:], 0.0)
            nc.vector.memset(hi[:], 2.0)

            def falsepos_tau(out_tile, clip_interior):
                """out = lo + (thW - S_lo)*(hi - lo)/(S_hi - S_lo), safeguarded."""
                nc.vector.tensor_tensor(out=scr0[:], in0=S_hi[:], in1=S_lo[:], op=ALU.subtract)
                nc.vector.reciprocal(out=scr1[:], in_=scr0[:])
                nc.vector.tensor_tensor(out=scr2[:], in0=thW[:], in1=S_lo[:], op=ALU.subtract)
                nc.vector.tensor_tensor(out=scr1[:], in0=scr2[:], in1=scr1[:], op=ALU.mult)
                nc.vector.tensor_tensor(out=scr2[:], in0=hi[:], in1=lo[:], op=ALU.subtract)
                nc.vector.tensor_tensor(out=scr1[:], in0=scr1[:], in1=scr2[:], op=ALU.mult)
                nc.vector.tensor_tensor(out=out_tile[:], in0=scr1[:], in1=lo[:], op=ALU.add)
                if clip_interior:
                    nc.vector.tensor_scalar_mul(scr1[:], scr2[:], 0.02)
                    nc.vector.tensor_tensor(out=scr2[:], in0=lo[:], in1=scr1[:], op=ALU.add)
                    nc.vector.tensor_tensor(out=out_tile[:], in0=out_tile[:], in1=scr2[:], op=ALU.max)
                    nc.vector.tensor_tensor(out=scr2[:], in0=hi[:], in1=scr1[:], op=ALU.subtract)
                    nc.vector.tensor_tensor(out=out_tile[:], in0=out_tile[:], in1=scr2[:], op=ALU.min)
                else:
                    nc.vector.tensor_tensor(out=out_tile[:], in0=out_tile[:], in1=lo[:], op=ALU.max)
                    nc.vector.tensor_tensor(out=out_tile[:], in0=out_tile[:], in1=hi[:], op=ALU.min)
                # guard: if S_hi - S_lo <= 0 fall back to midpoint
                nc.vector.tensor_scalar(out=pred_lt[:], in0=scr0[:], scalar1=0.0, scalar2=None,
                                        op0=ALU.is_le)
                nc.vector.tensor_tensor(out=scr2[:], in0=lo[:], in1=hi[:], op=ALU.add)
                nc.vector.tensor_scalar_mul(scr2[:], scr2[:], 0.5)
                nc.vector.copy_predicated(out=out_tile[:], mask=pred_lt[:], data=scr2[:])

            for k in range(n_bisect + n_fp):
                if k < n_bisect:
                    nc.vector.tensor_tensor(out=scr0[:], in0=lo[:], in1=hi[:], op=ALU.add)
                    nc.vector.tensor_scalar_mul(tau[:], scr0[:], 0.5)
                else:
                    falsepos_tau(tau, clip_interior=True)
                # round 0 fuses the t-build into the probe stream
                probe(tau, with_build=(k == 0))
                # bracket update: DVE-lane columns first (their accums land
                # before the ACT accumulator tail, so DVE does useful work
                # instead of stalling on S_act)
                nc.vector.tensor_tensor(out=pred_ge[:, n_act:], in0=S_dve[:],
                                        in1=thW[:, n_act:], op=ALU.is_ge)
                nc.vector.tensor_tensor(out=pred_lt[:, n_act:], in0=S_dve[:],
                                        in1=thW[:, n_act:], op=ALU.is_lt)
                nc.vector.copy_predicated(out=hi[:, n_act:], mask=pred_ge[:, n_act:], data=tau[:, n_act:])
                nc.vector.copy_predicated(out=lo[:, n_act:], mask=pred_lt[:, n_act:], data=tau[:, n_act:])
                nc.vector.copy_predicated(out=S_hi[:, n_act:], mask=pred_ge[:, n_act:], data=S_dve[:])
                nc.vector.copy_predicated(out=S_lo[:, n_act:], mask=pred_lt[:, n_act:], data=S_dve[:])
                nc.vector.tensor_tensor(out=pred_ge[:, :n_act], in0=S_act[:],
                                        in1=thW[:, :n_act], op=ALU.is_ge)
                nc.vector.tensor_tensor(out=pred_lt[:, :n_act], in0=S_act[:],
                                        in1=thW[:, :n_act], op=ALU.is_lt)
                nc.vector.copy_predicated(out=hi[:, :n_act], mask=pred_ge[:, :n_act], data=tau[:, :n_act])
                nc.vector.copy_predicated(out=lo[:, :n_act], mask=pred_lt[:, :n_act], data=tau[:, :n_act])
                nc.vector.copy_predicated(out=S_hi[:, :n_act], mask=pred_ge[:, :n_act], data=S_act[:])
                nc.vector.copy_predicated(out=S_lo[:, :n_act], mask=pred_lt[:, :n_act], data=S_act[:])

            # ---- final computed candidate (no probe) ----
            cand = st("cand")
            falsepos_tau(cand, clip_interior=False)
            if infguard:
                infs = st("infs")
                nc.vector.memset(infs[:], float("inf"))
                nc.vector.tensor_scalar(out=pred_ge[:], in0=hi[:], scalar1=2.0, scalar2=None,
                                        op0=ALU.is_ge)
                nc.vector.copy_predicated(out=cand[:], mask=pred_ge[:], data=infs[:])

            nc.sync.dma_start(out=out_loc[:, :], in_=cand[:])

    nc.compile()
    return nc


_NC_CACHE = None


def _host_prep(input_spikes, input_weights, input_delays, thresholds):
    s16 = np.ascontiguousarray(input_spikes, dtype=np.float16)
    w16 = np.asarray(input_weights, dtype=np.float16)
    d16 = np.asarray(input_delays, dtype=np.float16)
    th = np.asarray(thresholds, dtype=np.float32)
    sb = s16.astype(np.float32)
    wb = w16.astype(np.float32)
    db = d16.astype(np.float32)
    thw = (th[None, :] + (wb * db).sum(axis=0, dtype=np.float32)[None, :]
           + sb @ wb).astype(np.float32)
    slo = (thw - th[None, :]).astype(np.float32)                       # S(0)
    shi = np.broadcast_to(2.0 * wb.sum(axis=0, dtype=np.float32),      # S(2)
                          thw.shape).astype(np.float32)
    wT = np.ascontiguousarray(w16.T)
    dT = np.ascontiguousarray(d16.T)
    return s16, wT, dT, thw, slo, shi


def _to_state_layout(x_bloc_post):
    """[B_LOC, POST] -> [128, NCOL] with col = b*JB + jb, j = jb*128 + p."""
    x = np.asarray(x_bloc_post).reshape(B_LOC, JB, 128)        # [b, jb, p]
    return np.ascontiguousarray(x.transpose(2, 0, 1).reshape(128, NCOL))


def _from_state_layout(x_128_ncol):
    """[128, NCOL] -> [B_LOC, POST]."""
    x = np.asarray(x_128_ncol).reshape(128, B_LOC, JB)         # [p, b, jb]
    return np.ascontiguousarray(x.transpose(1, 2, 0).reshape(B_LOC, POST))


def kernel(input_spikes, input_weights, input_delays, thresholds):
    global _NC_CACHE
    if _NC_CACHE is None:
        _NC_CACHE = _build()
    nc = _NC_CACHE

    s16, wT, dT, thw, slo, shi = _host_prep(
        input_spikes, input_weights, input_delays, thresholds)

    in_maps = [
        dict(dT=dT, wT=wT,
             s_loc=np.ascontiguousarray(s16[k * B_LOC:(k + 1) * B_LOC]),
             thw_in=_to_state_layout(thw[k * B_LOC:(k + 1) * B_LOC]),
             slo_in=_to_state_layout(slo[k * B_LOC:(k + 1) * B_LOC]),
             shi_in=_to_state_layout(shi[k * B_LOC:(k + 1) * B_LOC]))
        for k in range(N_CORES)
    ]
    res = run_bass_kernel_spmd(nc, in_maps, core_ids=list(range(N_CORES)))
    out = np.concatenate([_from_state_layout(r["out_loc"]) for r in res.results],
                         axis=0)
    return out.astype(np.float32)


if __name__ == "__main__":
    rng = np.random.default_rng(0)
    s = rng.uniform(0, 1, (B, PRE)).astype(np.float32)
    w = (rng.normal(0, 1, (PRE, POST)) * 0.1 + 0.05).astype(np.float32)
    d = rng.uniform(0, 1, (PRE, POST)).astype(np.float32)
    th = np.ones(POST, np.float32)
    out = kernel(s, w, d, th)
    print("out", out.shape, out.dtype, np.percentile(out[np.isfinite(out)], [0, 50, 100]))


# revision 35
# speedup vs baseline: 1.1989x; 1.0089x over previous
"""Trainium2 Bass kernel for nn_EqualtimeLayer (spiking-neuron time-to-first-spike).

Math: for each (batch b, postsyn j) the output is the earliest T where
    f(T) = sum_i w[i,j] * relu(T - t[i,j]) >= theta_j,   t[i,j] = s[b,i] + d[i,j]
(first upward threshold crossing of the linear-PSP membrane potential; equivalent
to the reference's sort+cumsum+first-valid-window computation).

Device algorithm (no sort needed):
    S(tau) = sum_i w*max(t,tau)
    f(tau) >= theta  <=>  S(tau) >= thW := theta + sum_i w*t
    -> 4 rounds of bisection on [0,2], then 2 false-position probes, then a
       final computed false-position candidate. The bracket endpoints' S values
       start analytically known: S(0) = sum w*t = thW - theta, S(2) = 2*sum w
       (all t < 2), so false position needs no slope probes.

Each probe is evaluated with a two-lane engine split (the DVE fused
scalar_tensor_tensor runs at 1x only; tensor_scalar hits the 4x perf mode):
  - ACT-lane columns (24 = batches 0-2): DVE tensor_scalar(max) [4x] into the
    two halves of a [128, 2048] pair tile, one tensor_tensor(mult) [2x] over
    the pair (amortizes the per-op init), and the otherwise-idle Activation
    engine accumulates each half (activation Copy with accum_out).
  - DVE-lane columns (8 = batch 3): single fused scalar_tensor_tensor (1x).
Data is fp16 (DVE 2x/4x modes need 2-byte dtypes); state/accums are fp32. thW
is computed on the host from the same fp16-rounded inputs for consistency.

Sharding: data-parallel over batch, 4 batches per core on 8 cores. Weights and
delays are transposed once on the host (j-major layout) so probes are
per-partition-scalar ops with j on partitions and i on the free axis.
"""

import numpy as np

import concourse.bacc as bacc
import concourse.mybir as mybir
import concourse.tile as tile
from concourse.bass_utils import run_bass_kernel_spmd

F32 = mybir.dt.float32
F16 = mybir.dt.float16
U8 = mybir.dt.uint8
ALU = mybir.AluOpType
ACTF = mybir.ActivationFunctionType

B, PRE, POST = 32, 1024, 1024
N_CORES = 8
B_LOC = B // N_CORES          # 4 batches per core
JB = POST // 128              # 8 j-blocks of 128 partitions
NCOL = B_LOC * JB             # 32 state columns, col = b*JB + jb
R_BISECT = 4                  # bisection rounds
R_FALSEPOS = 2                # false-position probe rounds
N_ACT = 22                    # columns on the ACT lane (pairs; rest DVE-fused)


def _build(n_bisect=R_BISECT, n_fp=R_FALSEPOS, infguard=True):
    nc = bacc.Bacc("TRN2", target_bir_lowering=False, debug=False)

    dT = nc.dram_tensor("dT", [POST, PRE], F16, kind="ExternalInput")      # d transposed [j, i]
    wT = nc.dram_tensor("wT", [POST, PRE], F16, kind="ExternalInput")      # w transposed [j, i]
    s_loc = nc.dram_tensor("s_loc", [B_LOC, PRE], F16, kind="ExternalInput")
    # state-layout inputs/outputs: [128, NCOL] with col = b*JB + jb, j = jb*128+p
    # (host pre-/post-permutes; keeps every DMA a clean contiguous 2D copy)
    thw_in = nc.dram_tensor("thw_in", [128, NCOL], F32, kind="ExternalInput")
    slo_in = nc.dram_tensor("slo_in", [128, NCOL], F32, kind="ExternalInput")   # S(0) = thW - theta
    shi_in = nc.dram_tensor("shi_in", [128, NCOL], F32, kind="ExternalInput")   # S(2) = 2*sum w
    out_loc = nc.dram_tensor("out_loc", [128, NCOL], F32, kind="ExternalOutput")

    n_act = N_ACT

    with tile.TileContext(nc) as tc:
        with (
            tc.tile_pool(name="big", bufs=1) as big,
            tc.tile_pool(name="mat", bufs=1) as mat,
            tc.tile_pool(name="midp", bufs=2) as midp,
            tc.tile_pool(name="prodp", bufs=6) as prodp,
            tc.tile_pool(name="small", bufs=1) as small,
        ):
            # ---- load d^T and w^T, build t^T[b,jb] = d^T + s[b] ----
            # sync queue: dT0, sreps, dT1..7 (build-order critical path);
            # scalar queue: all wT + state tiles in parallel.
            dw = [mat.tile([128, PRE], F16, tag=f"dw{jb}", name=f"dT{jb}") for jb in range(JB)]
            sreps = []
            for b in range(B_LOC):
                srep = midp.tile([128, PRE], F16, tag=f"srep{b % 2}", name=f"srep{b}")
                nc.scalar.dma_start(out=srep[:], in_=s_loc[b:b + 1, :].partition_broadcast(128))
                sreps.append(srep)
            for jb in range(JB):
                nc.sync.dma_start(out=dw[jb][:], in_=dT[jb * 128:(jb + 1) * 128, :])

            # all w slices in ONE tile so a product op can span adjacent jb's
            w_all = mat.tile([128, JB * PRE], F16, tag="w_all", name="w_all")
            for jb in range(JB):
                nc.scalar.dma_start(out=w_all[:, jb * PRE:(jb + 1) * PRE],
                                    in_=wT[jb * 128:(jb + 1) * 128, :])

            # flat 2D t tiles, except ACT-lane batches keep per-b pair tiles
            tT = {}
            for b in range(B_LOC):
                for jb in range(JB):
                    tT[(b, jb)] = big.tile([128, PRE], F16, tag=f"tT{b}_{jb}",
                                           name=f"tT{b}_{jb}")

            # ---- per-column state, col = b*JB + jb ----
            def st(tag, dt=F32, ncol=NCOL):
                return small.tile([128, ncol], dt, tag=tag, name=tag)

            lo, hi, tau, thW = st("lo"), st("hi"), st("tau"), st("thW")
            S_lo, S_hi = st("S_lo"), st("S_hi")
            # S is split by accumulating engine to keep single-writer tiles
            S_act = st("S_act", ncol=n_act)
            S_dve = st("S_dve", ncol=NCOL - n_act)
            pred_ge, pred_lt = st("pge", U8), st("plt", U8)
            scr0, scr1, scr2 = st("scr0"), st("scr1"), st("scr2")
            dump = midp.tile([128, PRE], F16, tag="dump", name="dump")

            QW = 2  # ACT-lane columns grouped per product op

            def probe_group(b, jbq, scalar_tile):
                """ACT-lane probe of cols (b, QW*jbq .. QW*jbq+QW-1): QW 4x
                tensor_scalar maxes into a [128, QW*PRE] group, one 2x
                tensor_tensor mult over the group, QW ACT accumulations."""
                jbs = [QW * jbq + i for i in range(QW)]
                cols = [b * JB + jb for jb in jbs]
                mp = midp.tile([128, QW * PRE], F16, tag=f"mp{(b + jbq) % 2}", name="mp")
                for i, (jb, c) in enumerate(zip(jbs, cols)):
                    nc.vector.tensor_scalar(
                        out=mp[:, i * PRE:(i + 1) * PRE], in0=tT[(b, jb)][:],
                        scalar1=scalar_tile[:, c:c + 1], scalar2=None, op0=ALU.max)
                prod = prodp.tile([128, QW * PRE], F16, tag="prod", name="prod")
                nc.vector.tensor_tensor(
                    out=prod[:], in0=mp[:],
                    in1=w_all[:, jbs[0] * PRE:(jbs[-1] + 1) * PRE], op=ALU.mult)
                for i, c in enumerate(cols):
                    nc.scalar.activation(
                        out=dump[:], in_=prod[:, i * PRE:(i + 1) * PRE],
                        func=ACTF.Copy, scale=1.0,
                        accum_out=S_act[:, c:c + 1])

            def probe_stt(b, jb, scalar_tile):
                col = b * JB + jb
                scratch = prodp.tile([128, PRE], F16, tag="sc16", name="scratch")
                nc.vector.scalar_tensor_tensor(
                    out=scratch[:],
                    in0=tT[(b, jb)][:],
                    scalar=scalar_tile[:, col:col + 1],
                    in1=w_all[:, jb * PRE:(jb + 1) * PRE],
                    op0=ALU.max, op1=ALU.mult,
                    accum_out=S_dve[:, col - n_act:col - n_act + 1])

            def probe(scalar_tile, with_build=False):
                for jbq in range(JB // QW):
                    for b in range(B_LOC):
                        if with_build:
                            for jb in range(QW * jbq, QW * jbq + QW):
                                nc.vector.tensor_tensor(
                                    out=tT[(b, jb)][:], in0=dw[jb][:],
                                    in1=sreps[b][:], op=ALU.add)
                        if b * JB + QW * jbq + QW <= n_act:
                            probe_group(b, jbq, scalar_tile)
                        else:
                            for jb in range(QW * jbq, QW * jbq + QW):
                                probe_stt(b, jb, scalar_tile)

            # state-layout loads (already permuted on the host)
            for (dram, sb_tile) in ((thw_in, thW), (slo_in, S_lo), (shi_in, S_hi)):
                nc.scalar.dma_start(out=sb_tile[:], in_=dram[:, :])

            nc.vector.memset(lo[:], 0.0)
            nc.vector.memset(hi[:], 2.0)

            def falsepos_tau(out_tile, clip_interior):
                """out = lo + (thW - S_lo)*(hi - lo)/(S_hi - S_lo), safeguarded."""
                nc.vector.tensor_tensor(out=scr0[:], in0=S_hi[:], in1=S_lo[:], op=ALU.subtract)
                nc.vector.reciprocal(out=scr1[:], in_=scr0[:])
                nc.vector.tensor_tensor(out=scr2[:], in0=thW[:], in1=S_lo[:], op=ALU.subtract)
                nc.vector.tensor_tensor(out=scr1[:], in0=scr2[:], in1=scr1[:], op=ALU.mult)
                nc.vector.tensor_tensor(out=scr2[:], in0=hi[:], in1=lo[:], op=ALU.subtract)
                nc.vector.tensor_tensor(out=scr1[:], in0=scr1[:], in1=scr2[:], op=ALU.mult)
                nc.vector.tensor_tensor(out=out_tile[:], in0=scr1[:], in1=lo[:], op=ALU.add)
                if clip_interior:
                    nc.vector.tensor_scalar_mul(scr1[:], scr2[:], 0.02)
                    nc.vector.tensor_tensor(out=scr2[:], in0=lo[:], in1=scr1[:], op=ALU.add)
                    nc.vector.tensor_tensor(out=out_tile[:], in0=out_tile[:], in1=scr2[:], op=ALU.max)
                    nc.vector.tensor_tensor(out=scr2[:], in0=hi[:], in1=scr1[:], op=ALU.subtract)
                    nc.vector.tensor_tensor(out=out_tile[:], in0=out_tile[:], in1=scr2[:], op=ALU.min)
                else:
                    nc.vector.tensor_tensor(out=out_tile[:], in0=out_tile[:], in1=lo[:], op=ALU.max)
                    nc.vector.tensor_tensor(out=out_tile[:], in0=out_tile[:], in1=hi[:], op=ALU.min)
                # guard: if S_hi - S_lo <= 0 fall back to midpoint
                nc.vector.tensor_scalar(out=pred_lt[:], in0=scr0[:], scalar1=0.0, scalar2=None,
                                        op0=ALU.is_le)
                nc.vector.tensor_tensor(out=scr2[:], in0=lo[:], in1=hi[:], op=ALU.add)
                nc.vector.tensor_scalar_mul(scr2[:], scr2[:], 0.5)
                nc.vector.copy_predicated(out=out_tile[:], mask=pred_lt[:], data=scr2[:])

            for k in range(n_bisect + n_fp):
                if k < n_bisect:
                    nc.vector.tensor_tensor(out=scr0[:], in0=lo[:], in1=hi[:], op=ALU.add)
                    nc.vector.tensor_scalar_mul(tau[:], scr0[:], 0.5)
                else:
                    falsepos_tau(tau, clip_interior=False)
                # round 0 fuses the t-build into the probe stream
                probe(tau, with_build=(k == 0))
                # bracket update: DVE-lane columns first (their accums land
                # before the ACT accumulator tail, so DVE does useful work
                # instead of stalling on S_act)
                nc.vector.tensor_tensor(out=pred_ge[:, n_act:], in0=S_dve[:],
                                        in1=thW[:, n_act:], op=ALU.is_ge)
                nc.vector.tensor_tensor(out=pred_lt[:, n_act:], in0=S_dve[:],
                                        in1=thW[:, n_act:], op=ALU.is_lt)
                nc.vector.copy_predicated(out=hi[:, n_act:], mask=pred_ge[:, n_act:], data=tau[:, n_act:])
                nc.vector.copy_predicated(out=lo[:, n_act:], mask=pred_lt[:, n_act:], data=tau[:, n_act:])
                nc.vector.copy_predicated(out=S_hi[:, n_act:], mask=pred_ge[:, n_act:], data=S_dve[:])
                nc.vector.copy_predicated(out=S_lo[:, n_act:], mask=pred_lt[:, n_act:], data=S_dve[:])
                nc.vector.tensor_tensor(out=pred_ge[:, :n_act], in0=S_act[:],
                                        in1=thW[:, :n_act], op=ALU.is_ge)
                nc.vector.tensor_tensor(out=pred_lt[:, :n_act], in0=S_act[:],
                                        in1=thW[:, :n_act], op=ALU.is_lt)
                nc.vector.copy_predicated(out=hi[:, :n_act], mask=pred_ge[:, :n_act], data=tau[:, :n_act])
                nc.vector.copy_predicated(out=lo[:, :n_act], mask=pred_lt[:, :n_act], data=tau[:, :n_act])
                nc.vector.copy_predicated(out=S_hi[:, :n_act], mask=pred_ge[:, :n_act], data=S_act[:])
                nc.vector.copy_predicated(out=S_lo[:, :n_act], mask=pred_lt[:, :n_act], data=S_act[:])

            # ---- final computed candidate (no probe) ----
            cand = st("cand")
            falsepos_tau(cand, clip_interior=False)
            if infguard:
                infs = st("infs")
                nc.vector.memset(infs[:], float("inf"))
                nc.vector.tensor_scalar(out=pred_ge[:], in0=hi[:], scalar1=2.0, scalar2=None,
                                        op0=ALU.is_ge)
                nc.vector.copy_predicated(out=cand[:], mask=pred_ge[:], data=infs[:])

            nc.sync.dma_start(out=out_loc[:, :], in_=cand[:])

    nc.compile()
    return nc


_NC_CACHE = None


def _host_prep(input_spikes, input_weights, input_delays, thresholds):
    s16 = np.ascontiguousarray(input_spikes, dtype=np.float16)
    w16 = np.asarray(input_weights, dtype=np.float16)
    d16 = np.asarray(input_delays, dtype=np.float16)
    th = np.asarray(thresholds, dtype=np.float32)
    sb = s16.astype(np.float32)
    wb = w16.astype(np.float32)
    db = d16.astype(np.float32)
    thw = (th[None, :] + (wb * db).sum(axis=0, dtype=np.float32)[None, :]
           + sb @ wb).astype(np.float32)
    slo = (thw - th[None, :]).astype(np.float32)                       # S(0)
    shi = np.broadcast_to(2.0 * wb.sum(axis=0, dtype=np.float32),      # S(2)
                          thw.shape).astype(np.float32)
    wT = np.ascontiguousarray(w16.T)
    dT = np.ascontiguousarray(d16.T)
    return s16, wT, dT, thw, slo, shi


def _to_state_layout(x_bloc_post):
    """[B_LOC, POST] -> [128, NCOL] with col = b*JB + jb, j = jb*128 + p."""
    x = np.asarray(x_bloc_post).reshape(B_LOC, JB, 128)        # [b, jb, p]
    return np.ascontiguousarray(x.transpose(2, 0, 1).reshape(128, NCOL))


def _from_state_layout(x_128_ncol):
    """[128, NCOL] -> [B_LOC, POST]."""
    x = np.asarray(x_128_ncol).reshape(128, B_LOC, JB)         # [p, b, jb]
    return np.ascontiguousarray(x.transpose(1, 2, 0).reshape(B_LOC, POST))


def kernel(input_spikes, input_weights, input_delays, thresholds):
    global _NC_CACHE
    if _NC_CACHE is None:
        _NC_CACHE = _build()
    nc = _NC_CACHE

    s16, wT, dT, thw, slo, shi = _host_prep(
        input_spikes, input_weights, input_delays, thresholds)

    in_maps = [
        dict(dT=dT, wT=wT,
             s_loc=np.ascontiguousarray(s16[k * B_LOC:(k + 1) * B_LOC]),
             thw_in=_to_state_layout(thw[k * B_LOC:(k + 1) * B_LOC]),
             slo_in=_to_state_layout(slo[k * B_LOC:(k + 1) * B_LOC]),
             shi_in=_to_state_layout(shi[k * B_LOC:(k + 1) * B_LOC]))
        for k in range(N_CORES)
    ]
    res = run_bass_kernel_spmd(nc, in_maps, core_ids=list(range(N_CORES)))
    out = np.concatenate([_from_state_layout(r["out_loc"]) for r in res.results],
                         axis=0)
    return out.astype(np.float32)


if __name__ == "__main__":
    rng = np.random.default_rng(0)
    s = rng.uniform(0, 1, (B, PRE)).astype(np.float32)
    w = (rng.normal(0, 1, (PRE, POST)) * 0.1 + 0.05).astype(np.float32)
    d = rng.uniform(0, 1, (PRE, POST)).astype(np.float32)
    th = np.ones(POST, np.float32)
    out = kernel(s, w, d, th)
    print("out", out.shape, out.dtype, np.percentile(out[np.isfinite(out)], [0, 50, 100]))
